# revision 5
# baseline (speedup 1.0000x reference)
"""AffinityEnergyLoss on 8 Trainium2 NeuronCores.

Sharding: core k handles (layer l = k // 4, batch b = k % 4) — one
(l, b) slab of the encoder attns (8 heads x 1025 x 1025, CLS cropped)
plus the matching slab of decoder attns (8 heads x 1024 x 1024).

Per core, for each of its 16 maps M (1024 x 1024, fp32):
    rowsum s = M @ 1          (DVE reduce / ACT activation accum_out)
    r = 1/s                   (DVE reciprocal)
    S += diag(r) @ M          (PE fp32r matmul, accumulated in PSUM
                               over all 16 maps; diag(r) built as eye*r)
so S = sum_m D_m M_m for the core's maps. Then per 128-row block:
    T = S^T                   (PE transpose via identity)
    Z = S @ [softmax(preds_b) | 1]   (exact fp32 PE matmul over T chunks)
Z (1024 x 22) is the core's partial of sum_m D_m M_m @ [P|1].

Host: affinity_raw_b = (Z_{l=0,b} + Z_{l=1,b}) / 32, row-normalize the
first 21 columns, then loss = sum(roi * |prob - affinity|) / N.
"""
import numpy as np

import concourse.bacc as bacc
import concourse.mybir as mybir
import concourse.tile as tile
from concourse.bass_utils import run_bass_kernel_spmd

F32 = mybir.dt.float32
F32R = mybir.dt.float32r
AX = mybir.AxisListType.X
ACTF = mybir.ActivationFunctionType

HEADS = 8
TOK = 1024
C = 21
PB = 128          # partition block
NBLK = TOK // PB  # 8

_NC = None


def _build_nc():
    nc = bacc.Bacc(None, target_bir_lowering=False)
    enc = nc.dram_tensor("enc", [HEADS, 1025, 1025], F32, kind="ExternalInput")
    dec = nc.dram_tensor("dec", [HEADS, TOK, TOK], F32, kind="ExternalInput")
    pt = nc.dram_tensor("pt", [TOK, C], F32, kind="ExternalInput")
    eye = nc.dram_tensor("eye", [PB, PB], F32, kind="ExternalInput")
    z = nc.dram_tensor("z", [TOK, C + 1], F32, kind="ExternalOutput")

    HCHUNK = 4  # heads per DMA chunk (2 MB loads)

    def _chunk_src(enc, dec, ib, c4):
        i0 = ib * PB
        if c4 < 2:
            h0 = c4 * HCHUNK
            return enc[h0 : h0 + HCHUNK, 1 + i0 : 1 + i0 + PB, 1:].transpose([1, 0, 2])
        h0 = (c4 - 2) * HCHUNK
        return dec[h0 : h0 + HCHUNK, i0 : i0 + PB, :].transpose([1, 0, 2])

    with tile.TileContext(nc) as tc:
        with (
            tc.tile_pool(name="const", bufs=1) as const,
            tc.tile_pool(name="stats", bufs=8) as stats,
            tc.tile_pool(name="big", bufs=6) as big,
            tc.tile_pool(name="spool", bufs=2) as spool,
            tc.tile_pool(name="zout", bufs=2) as zout,
            tc.tile_pool(name="psS", bufs=2, space="PSUM") as psS,
            tc.tile_pool(name="psT", bufs=2, space="PSUM") as psT,
            tc.tile_pool(name="psZ", bufs=2, space="PSUM") as psZ,
        ):
            # issue the first block's big loads before anything else
            chunk_tiles = {}
            for c4 in range(4):
                t = big.tile([PB, HCHUNK, TOK], F32R, tag="chunk")
                nc.gpsimd.dma_start(out=t[:], in_=_chunk_src(enc, dec, 0, c4))
                chunk_tiles[(0, c4)] = t

            eye_sb = const.tile([PB, PB], F32)
            nc.sync.dma_start(out=eye_sb[:], in_=eye[:])

            # PE warm-up: a dense burst of dummy bf16 matmuls trips the HAM
            # clock gate to 8/8 before the first real (fp32r) matmul arrives;
            # the steady-state matmul stream then keeps it warm.
            wu_a = const.tile([PB, PB], mybir.dt.bfloat16)
            nc.vector.memset(wu_a[:], 0.0)
            wu_ps = psT.tile([PB, PB], F32, tag="tp")
            for _ in range(100):
                nc.tensor.matmul(wu_ps[:], wu_a[:], wu_a[:], start=True, stop=True)

            # Pa = [softmax(pt) | 1] laid out [p, chunk, 22], j = chunk*128 + p
            pt_sb = const.tile([PB, NBLK, C], F32)
            nc.sync.dma_start(
                out=pt_sb[:], in_=pt.rearrange("(c p) n -> p c n", p=PB)
            )
            pa_sb = const.tile([PB, NBLK, C + 1], F32)
            for c in range(NBLK):
                negmx = stats.tile([PB, 1], F32, tag="negmx")
                nc.vector.reduce_max(negmx[:], pt_sb[:, c, :], axis=AX, negate=True)
                ssum = stats.tile([PB, 1], F32, tag="ssum")
                nc.scalar.activation(
                    pa_sb[:, c, 0:C],
                    pt_sb[:, c, :],
                    ACTF.Exp,
                    bias=negmx[:],
                    accum_out=ssum[:],
                )
                rs = stats.tile([PB, 1], F32, tag="rs")
                nc.vector.reciprocal(rs[:], ssum[:])
                nc.vector.tensor_scalar_mul(pa_sb[:, c, 0:C], pa_sb[:, c, 0:C], rs[:])
                nc.vector.memset(pa_sb[:, c, C : C + 1], 1.0)

            for ib in range(NBLK):
                i0 = ib * PB
                S_ps = psS.tile([PB, TOK], F32)
                for c4 in range(4):
                    t = chunk_tiles.pop((ib, c4), None)
                    if t is None:
                        t = big.tile([PB, HCHUNK, TOK], F32R, tag="chunk")
                        nc.gpsimd.dma_start(out=t[:], in_=_chunk_src(enc, dec, ib, c4))
                    for hm in range(HCHUNK):
                        m = c4 * HCHUNK + hm
                        src = t[:, hm, :]
                        s_m = stats.tile([PB, 1], F32, tag="s_m")
                        if m % 2 == 0:
                            nc.vector.reduce_sum(s_m[:], src.bitcast(F32), axis=AX)
                        else:
                            scr = spool.tile([PB, TOK], F32, tag="scr")
                            nc.scalar.activation(
                                scr[:], src.bitcast(F32), ACTF.Copy, accum_out=s_m[:]
                            )
                        r_m = stats.tile([PB, 1], F32, tag="r_m")
                        nc.vector.reciprocal(r_m[:], s_m[:])
                        dg = stats.tile([PB, PB], F32R, tag="dg")
                        nc.vector.tensor_scalar_mul(dg[:], eye_sb[:], r_m[:])
                        nc.tensor.matmul(
                            S_ps[:, 0:512], dg[:], src[:, 0:512],
                            start=(m == 0), stop=(m == 15),
                        )
                        nc.tensor.matmul(
                            S_ps[:, 512:1024], dg[:], src[:, 512:1024],
                            start=(m == 0), stop=(m == 15),
                        )

                S_sb = spool.tile([PB, TOK], F32, tag="S")
                nc.scalar.copy(out=S_sb[:], in_=S_ps[:])
                T_sb = spool.tile([PB, NBLK, PB], F32, tag="T")
                for jc in range(NBLK):
                    tp = psT.tile([PB, PB], F32)
                    nc.tensor.transpose(
                        tp[:], S_sb[:, jc * PB : (jc + 1) * PB], eye_sb[:]
                    )
                    nc.scalar.copy(out=T_sb[:, jc, :], in_=tp[:])
                z_ps = psZ.tile([PB, C + 1], F32)
                for jc in range(NBLK):
                    nc.tensor.matmul(
                        z_ps[:], T_sb[:, jc, :], pa_sb[:, jc, :],
                        start=(jc == 0), stop=(jc == NBLK - 1),
                    )
                z_sb = zout.tile([PB, C + 1], F32)
                nc.vector.tensor_copy(z_sb[:], z_ps[:])
                nc.sync.dma_start(out=z[i0 : i0 + PB, :], in_=z_sb[:])

    nc.compile()
    return nc


def _get_nc():
    global _NC
    if _NC is None:
        _NC = _build_nc()
    return _NC


def kernel(preds, low_feats, high_feats, unlabeled_ROIs, targets, attns, decode_attns):
    preds = np.asarray(preds, dtype=np.float32)
    attns = np.asarray(attns, dtype=np.float32)
    decode_attns = np.asarray(decode_attns, dtype=np.float32)
    roi = np.asarray(unlabeled_ROIs)

    bz = preds.shape[0]
    preds_t = np.ascontiguousarray(
        preds.reshape(bz, C, TOK).transpose(0, 2, 1)
    )  # (bz, 1024, 21)
    eye_np = np.eye(PB, dtype=np.float32)

    nc = _get_nc()
    in_maps = []
    for k in range(8):
        l, b = k // 4, k % 4
        in_maps.append(
            {
                "enc": np.ascontiguousarray(attns[l, b]),
                "dec": np.ascontiguousarray(decode_attns[l, b]),
                "pt": preds_t[b],
                "eye": eye_np,
            }
        )
    res = run_bass_kernel_spmd(nc, in_maps, core_ids=list(range(8)))
    zs = np.stack([res.results[k]["z"] for k in range(8)])  # (8, 1024, 22)

    # combine: affinity_raw_b = (Z_{l=0,b} + Z_{l=1,b}) / 32
    zb = zs.reshape(2, bz, TOK, C + 1).sum(axis=0) / 32.0
    aff = zb[:, :, :C]
    aff = aff / aff.sum(axis=-1, keepdims=True)

    # host softmax (matches jax.nn.softmax in f32)
    e = np.exp(preds_t - preds_t.max(axis=-1, keepdims=True))
    prob = e / e.sum(axis=-1, keepdims=True)  # (bz, 1024, 21)

    roi_f = roi.astype(np.float32).reshape(bz, TOK, 1)
    n_roi = roi_f.sum()
    loss = (roi_f * np.abs(prob - aff)).sum()
    if n_roi > 0:
        loss = loss / n_roi
    return np.asarray(loss, dtype=np.float32)


# revision 6
# speedup vs baseline: 1.0660x; 1.0660x over previous
"""AffinityEnergyLoss on 8 Trainium2 NeuronCores.

Sharding: core k handles (layer l = k // 4, batch b = k % 4) — one
(l, b) slab of the encoder attns (8 heads x 1025 x 1025, CLS cropped)
plus the matching slab of decoder attns (8 heads x 1024 x 1024).

Per core, for each of its 16 maps M (1024 x 1024, fp32):
    rowsum s = M @ 1          (DVE reduce / ACT activation accum_out)
    r = 1/s                   (DVE reciprocal)
    S += diag(r) @ M          (PE fp32r matmul, accumulated in PSUM
                               over all 16 maps; diag(r) built as eye*r)
so S = sum_m D_m M_m for the core's maps. Then per 128-row block:
    T = S^T                   (PE transpose via identity)
    Z = S @ [softmax(preds_b) | 1]   (exact fp32 PE matmul over T chunks)
Z (1024 x 22) is the core's partial of sum_m D_m M_m @ [P|1].

Host: affinity_raw_b = (Z_{l=0,b} + Z_{l=1,b}) / 32, row-normalize the
first 21 columns, then loss = sum(roi * |prob - affinity|) / N.
"""
import numpy as np

import concourse.bacc as bacc
import concourse.mybir as mybir
import concourse.tile as tile
from concourse.bass_utils import run_bass_kernel_spmd

F32 = mybir.dt.float32
F32R = mybir.dt.float32r
AX = mybir.AxisListType.X
ACTF = mybir.ActivationFunctionType

HEADS = 8
TOK = 1024
C = 21
PB = 128          # partition block
NBLK = TOK // PB  # 8

_NC = None


def _build_nc():
    nc = bacc.Bacc(None, target_bir_lowering=False)
    enc = nc.dram_tensor("enc", [HEADS, 1025, 1025], F32, kind="ExternalInput")
    dec = nc.dram_tensor("dec", [HEADS, TOK, TOK], F32, kind="ExternalInput")
    pt = nc.dram_tensor("pt", [TOK, C], F32, kind="ExternalInput")
    eye = nc.dram_tensor("eye", [PB, PB], F32, kind="ExternalInput")
    z = nc.dram_tensor("z", [TOK, C + 1], F32, kind="ExternalOutput")

    HCHUNK = 4  # heads per DMA chunk (2 MB loads)

    def _chunk_src(enc, dec, ib, c4):
        i0 = ib * PB
        if c4 < 2:
            h0 = c4 * HCHUNK
            return enc[h0 : h0 + HCHUNK, 1 + i0 : 1 + i0 + PB, 1:].transpose([1, 0, 2])
        h0 = (c4 - 2) * HCHUNK
        return dec[h0 : h0 + HCHUNK, i0 : i0 + PB, :].transpose([1, 0, 2])

    with tile.TileContext(nc) as tc:
        with (
            tc.tile_pool(name="const", bufs=1) as const,
            tc.tile_pool(name="stats", bufs=8) as stats,
            tc.tile_pool(name="big", bufs=6) as big,
            tc.tile_pool(name="spool", bufs=2) as spool,
            tc.tile_pool(name="zout", bufs=2) as zout,
            tc.tile_pool(name="psS", bufs=2, space="PSUM") as psS,
            tc.tile_pool(name="psT", bufs=2, space="PSUM") as psT,
            tc.tile_pool(name="psZ", bufs=2, space="PSUM") as psZ,
        ):
            # issue the first block's big loads before anything else
            chunk_tiles = {}
            for c4 in range(4):
                t = big.tile([PB, HCHUNK, TOK], F32R, tag="chunk")
                nc.gpsimd.dma_start(out=t[:], in_=_chunk_src(enc, dec, 0, c4))
                chunk_tiles[(0, c4)] = t

            eye_sb = const.tile([PB, PB], F32)
            nc.sync.dma_start(out=eye_sb[:], in_=eye[:])

            # PE warm-up: a dense burst of dummy bf16 matmuls trips the HAM
            # clock gate to 8/8 before the first real (fp32r) matmul arrives;
            # the steady-state matmul stream then keeps it warm.
            wu_a = const.tile([PB, PB], mybir.dt.bfloat16)
            nc.vector.memset(wu_a[:], 0.0)
            wu_ps = psT.tile([PB, PB], F32, tag="tp")
            for i in range(100):
                nc.tensor.matmul(
                    wu_ps[:], wu_a[:], wu_a[:], start=(i == 0), stop=(i == 99)
                )

            # Pa = [softmax(pt) | 1] laid out [p, chunk, 22], j = chunk*128 + p
            pt_sb = const.tile([PB, NBLK, C], F32)
            nc.sync.dma_start(
                out=pt_sb[:], in_=pt.rearrange("(c p) n -> p c n", p=PB)
            )
            pa_sb = const.tile([PB, NBLK, C + 1], F32)
            for c in range(NBLK):
                negmx = stats.tile([PB, 1], F32, tag="negmx")
                nc.vector.reduce_max(negmx[:], pt_sb[:, c, :], axis=AX, negate=True)
                ssum = stats.tile([PB, 1], F32, tag="ssum")
                nc.scalar.activation(
                    pa_sb[:, c, 0:C],
                    pt_sb[:, c, :],
                    ACTF.Exp,
                    bias=negmx[:],
                    accum_out=ssum[:],
                )
                rs = stats.tile([PB, 1], F32, tag="rs")
                nc.vector.reciprocal(rs[:], ssum[:])
                nc.vector.tensor_scalar_mul(pa_sb[:, c, 0:C], pa_sb[:, c, 0:C], rs[:])
                nc.vector.memset(pa_sb[:, c, C : C + 1], 1.0)

            for ib in range(NBLK):
                i0 = ib * PB
                S_ps = psS.tile([PB, TOK], F32)
                for c4 in range(4):
                    t = chunk_tiles.pop((ib, c4), None)
                    if t is None:
                        t = big.tile([PB, HCHUNK, TOK], F32R, tag="chunk")
                        nc.gpsimd.dma_start(out=t[:], in_=_chunk_src(enc, dec, ib, c4))
                    for hm in range(HCHUNK):
                        m = c4 * HCHUNK + hm
                        src = t[:, hm, :]
                        s_m = stats.tile([PB, 1], F32, tag="s_m")
                        if m % 2 == 0:
                            nc.vector.reduce_sum(s_m[:], src.bitcast(F32), axis=AX)
                        else:
                            scr = spool.tile([PB, TOK], F32, tag="scr")
                            nc.scalar.activation(
                                scr[:], src.bitcast(F32), ACTF.Copy, accum_out=s_m[:]
                            )
                        r_m = stats.tile([PB, 1], F32, tag="r_m")
                        nc.vector.reciprocal(r_m[:], s_m[:])
                        dg = stats.tile([PB, PB], F32R, tag="dg")
                        nc.vector.tensor_scalar_mul(dg[:], eye_sb[:], r_m[:])
                        nc.tensor.matmul(
                            S_ps[:, 0:512], dg[:], src[:, 0:512],
                            start=(m == 0), stop=(m == 15),
                        )
                        nc.tensor.matmul(
                            S_ps[:, 512:1024], dg[:], src[:, 512:1024],
                            start=(m == 0), stop=(m == 15),
                        )

                S_sb = spool.tile([PB, TOK], F32, tag="S")
                nc.scalar.copy(out=S_sb[:], in_=S_ps[:])
                T_sb = spool.tile([PB, NBLK, PB], F32, tag="T")
                for jc in range(NBLK):
                    tp = psT.tile([PB, PB], F32)
                    nc.tensor.transpose(
                        tp[:], S_sb[:, jc * PB : (jc + 1) * PB], eye_sb[:]
                    )
                    nc.scalar.copy(out=T_sb[:, jc, :], in_=tp[:])
                z_ps = psZ.tile([PB, C + 1], F32)
                for jc in range(NBLK):
                    nc.tensor.matmul(
                        z_ps[:], T_sb[:, jc, :], pa_sb[:, jc, :],
                        start=(jc == 0), stop=(jc == NBLK - 1),
                    )
                z_sb = zout.tile([PB, C + 1], F32)
                nc.vector.tensor_copy(z_sb[:], z_ps[:])
                nc.sync.dma_start(out=z[i0 : i0 + PB, :], in_=z_sb[:])

    nc.compile()
    return nc


def _get_nc():
    global _NC
    if _NC is None:
        _NC = _build_nc()
    return _NC


def kernel(preds, low_feats, high_feats, unlabeled_ROIs, targets, attns, decode_attns):
    preds = np.asarray(preds, dtype=np.float32)
    attns = np.asarray(attns, dtype=np.float32)
    decode_attns = np.asarray(decode_attns, dtype=np.float32)
    roi = np.asarray(unlabeled_ROIs)

    bz = preds.shape[0]
    preds_t = np.ascontiguousarray(
        preds.reshape(bz, C, TOK).transpose(0, 2, 1)
    )  # (bz, 1024, 21)
    eye_np = np.eye(PB, dtype=np.float32)

    nc = _get_nc()
    in_maps = []
    for k in range(8):
        l, b = k // 4, k % 4
        in_maps.append(
            {
                "enc": np.ascontiguousarray(attns[l, b]),
                "dec": np.ascontiguousarray(decode_attns[l, b]),
                "pt": preds_t[b],
                "eye": eye_np,
            }
        )
    res = run_bass_kernel_spmd(nc, in_maps, core_ids=list(range(8)))
    zs = np.stack([res.results[k]["z"] for k in range(8)])  # (8, 1024, 22)

    # combine: affinity_raw_b = (Z_{l=0,b} + Z_{l=1,b}) / 32
    zb = zs.reshape(2, bz, TOK, C + 1).sum(axis=0) / 32.0
    aff = zb[:, :, :C]
    aff = aff / aff.sum(axis=-1, keepdims=True)

    # host softmax (matches jax.nn.softmax in f32)
    e = np.exp(preds_t - preds_t.max(axis=-1, keepdims=True))
    prob = e / e.sum(axis=-1, keepdims=True)  # (bz, 1024, 21)

    roi_f = roi.astype(np.float32).reshape(bz, TOK, 1)
    n_roi = roi_f.sum()
    loss = (roi_f * np.abs(prob - aff)).sum()
    if n_roi > 0:
        loss = loss / n_roi
    return np.asarray(loss, dtype=np.float32)


# revision 7
# speedup vs baseline: 1.2140x; 1.1389x over previous
"""AffinityEnergyLoss on 8 Trainium2 NeuronCores.

Sharding: core k handles (layer l = k // 4, batch b = k % 4) — one
(l, b) slab of the encoder attns (8 heads x 1025 x 1025, CLS cropped)
plus the matching slab of decoder attns (8 heads x 1024 x 1024).

Per core, for each of its 16 maps M (1024 x 1024, fp32):
    rowsum s = M @ 1          (DVE reduce / ACT activation accum_out)
    r = 1/s                   (DVE reciprocal)
    S += diag(r) @ M          (PE fp32r matmul, accumulated in PSUM
                               over all 16 maps; diag(r) built as eye*r)
so S = sum_m D_m M_m for the core's maps. Then per 128-row block:
    T = S^T                   (PE transpose via identity)
    Z = S @ [softmax(preds_b) | 1]   (exact fp32 PE matmul over T chunks)
Z (1024 x 22) is the core's partial of sum_m D_m M_m @ [P|1].

Host: affinity_raw_b = (Z_{l=0,b} + Z_{l=1,b}) / 32, row-normalize the
first 21 columns, then loss = sum(roi * |prob - affinity|) / N.
"""
import numpy as np

import concourse.bacc as bacc
import concourse.mybir as mybir
import concourse.tile as tile
from concourse.bass_utils import run_bass_kernel_spmd

F32 = mybir.dt.float32
F32R = mybir.dt.float32r
AX = mybir.AxisListType.X
ACTF = mybir.ActivationFunctionType

HEADS = 8
TOK = 1024
C = 21
PB = 128          # partition block
NBLK = TOK // PB  # 8

_NC = None


def _build_nc():
    nc = bacc.Bacc(None, target_bir_lowering=False)
    enc = nc.dram_tensor("enc", [HEADS, 1025, 1025], F32, kind="ExternalInput")
    dec = nc.dram_tensor("dec", [HEADS, TOK, TOK], F32, kind="ExternalInput")
    pt = nc.dram_tensor("pt", [TOK, C], F32, kind="ExternalInput")
    eye = nc.dram_tensor("eye", [PB, PB], F32, kind="ExternalInput")
    z = nc.dram_tensor("z", [NBLK, C, PB], F32, kind="ExternalOutput")

    HCHUNK = 4  # heads per DMA chunk (2 MB loads)

    def _chunk_src(enc, dec, ib, c4):
        i0 = ib * PB
        if c4 < 2:
            h0 = c4 * HCHUNK
            return enc[h0 : h0 + HCHUNK, 1 + i0 : 1 + i0 + PB, 1:].transpose([1, 0, 2])
        h0 = (c4 - 2) * HCHUNK
        return dec[h0 : h0 + HCHUNK, i0 : i0 + PB, :].transpose([1, 0, 2])

    with tile.TileContext(nc) as tc:
        with (
            tc.tile_pool(name="const", bufs=1) as const,
            tc.tile_pool(name="stats", bufs=8) as stats,
            tc.tile_pool(name="big", bufs=6) as big,
            tc.tile_pool(name="spool", bufs=2) as spool,
            tc.tile_pool(name="zout", bufs=2) as zout,
            tc.tile_pool(name="psS", bufs=2, space="PSUM") as psS,
            tc.tile_pool(name="psT", bufs=2, space="PSUM") as psT,
            tc.tile_pool(name="psZ", bufs=2, space="PSUM") as psZ,
        ):
            # issue the first block's big loads before anything else
            chunk_tiles = {}
            for c4 in range(4):
                t = big.tile([PB, HCHUNK, TOK], F32R, tag="chunk")
                nc.gpsimd.dma_start(out=t[:], in_=_chunk_src(enc, dec, 0, c4))
                chunk_tiles[(0, c4)] = t

            eye_sb = const.tile([PB, PB], F32)
            nc.sync.dma_start(out=eye_sb[:], in_=eye[:])

            # Pa = [softmax(pt) | 1] laid out [p, chunk, 22], j = chunk*128 + p
            pt_sb = const.tile([PB, NBLK, C], F32)
            nc.sync.dma_start(
                out=pt_sb[:], in_=pt.rearrange("(c p) n -> p c n", p=PB)
            )
            pa_sb = const.tile([PB, NBLK, C], F32R)
            for c in range(NBLK):
                negmx = stats.tile([PB, 1], F32, tag="negmx")
                nc.vector.reduce_max(negmx[:], pt_sb[:, c, :], axis=AX, negate=True)
                ssum = stats.tile([PB, 1], F32, tag="ssum")
                ex = stats.tile([PB, C], F32, tag="ex")
                nc.scalar.activation(
                    ex[:],
                    pt_sb[:, c, :],
                    ACTF.Exp,
                    bias=negmx[:],
                    accum_out=ssum[:],
                )
                rs = stats.tile([PB, 1], F32, tag="rs")
                nc.vector.reciprocal(rs[:], ssum[:])
                nc.vector.tensor_scalar_mul(pa_sb[:, c, :], ex[:], rs[:])

            for ib in range(NBLK):
                i0 = ib * PB
                S_ps = psS.tile([PB, TOK], F32)
                for c4 in range(4):
                    t = chunk_tiles.pop((ib, c4), None)
                    if t is None:
                        t = big.tile([PB, HCHUNK, TOK], F32R, tag="chunk")
                        nc.gpsimd.dma_start(out=t[:], in_=_chunk_src(enc, dec, ib, c4))
                    for hm in range(HCHUNK):
                        m = c4 * HCHUNK + hm
                        src = t[:, hm, :]
                        s_m = stats.tile([PB, 1], F32, tag="s_m")
                        if m % 2 == 0:
                            nc.vector.reduce_sum(s_m[:], src.bitcast(F32), axis=AX)
                        else:
                            scr = spool.tile([PB, TOK], F32, tag="scr")
                            nc.scalar.activation(
                                scr[:], src.bitcast(F32), ACTF.Copy, accum_out=s_m[:]
                            )
                        r_m = stats.tile([PB, 1], F32, tag="r_m")
                        nc.vector.reciprocal(r_m[:], s_m[:])
                        dg = stats.tile([PB, PB], F32R, tag="dg")
                        nc.vector.tensor_scalar_mul(dg[:], eye_sb[:], r_m[:])
                        nc.tensor.matmul(
                            S_ps[:, 0:512], dg[:], src[:, 0:512],
                            start=(m == 0), stop=(m == 15),
                        )
                        nc.tensor.matmul(
                            S_ps[:, 512:1024], dg[:], src[:, 512:1024],
                            start=(m == 0), stop=(m == 15),
                        )

                S_sb = spool.tile([PB, TOK], F32, tag="S")
                nc.scalar.copy(out=S_sb[:], in_=S_ps[:])
                T_sb = spool.tile([PB, NBLK, PB], F32R, tag="T")
                for jc in range(NBLK):
                    tp = psT.tile([PB, PB], F32)
                    nc.tensor.transpose(
                        tp[:], S_sb[:, jc * PB : (jc + 1) * PB], eye_sb[:]
                    )
                    nc.scalar.copy(out=T_sb[:, jc, :], in_=tp[:])
                z_ps = psZ.tile([C, PB], F32)
                for jc in range(NBLK):
                    nc.tensor.matmul(
                        z_ps[:], pa_sb[:, jc, :], T_sb[:, jc, :],
                        start=(jc == 0), stop=(jc == NBLK - 1),
                    )
                z_sb = zout.tile([C, PB], F32)
                nc.vector.tensor_copy(z_sb[:], z_ps[:])
                nc.sync.dma_start(out=z[ib, :, :], in_=z_sb[:])

    nc.compile()
    return nc


def _get_nc():
    global _NC
    if _NC is None:
        _NC = _build_nc()
    return _NC


def kernel(preds, low_feats, high_feats, unlabeled_ROIs, targets, attns, decode_attns):
    preds = np.asarray(preds, dtype=np.float32)
    attns = np.asarray(attns, dtype=np.float32)
    decode_attns = np.asarray(decode_attns, dtype=np.float32)
    roi = np.asarray(unlabeled_ROIs)

    bz = preds.shape[0]
    preds_t = np.ascontiguousarray(
        preds.reshape(bz, C, TOK).transpose(0, 2, 1)
    )  # (bz, 1024, 21)
    eye_np = np.eye(PB, dtype=np.float32)

    nc = _get_nc()
    in_maps = []
    for k in range(8):
        l, b = k // 4, k % 4
        in_maps.append(
            {
                "enc": np.ascontiguousarray(attns[l, b]),
                "dec": np.ascontiguousarray(decode_attns[l, b]),
                "pt": preds_t[b],
                "eye": eye_np,
            }
        )
    res = run_bass_kernel_spmd(nc, in_maps, core_ids=list(range(8)))
    # z per core: (NBLK, C, PB) holding Z^T per block -> (1024, 21)
    zs = np.stack(
        [
            res.results[k]["z"].transpose(0, 2, 1).reshape(TOK, C)
            for k in range(8)
        ]
    )

    # combine: affinity_raw_b = (Z_{l=0,b} + Z_{l=1,b}) / 32
    zb = zs.reshape(2, bz, TOK, C).sum(axis=0) / 32.0
    aff = zb / zb.sum(axis=-1, keepdims=True)

    # host softmax (matches jax.nn.softmax in f32)
    e = np.exp(preds_t - preds_t.max(axis=-1, keepdims=True))
    prob = e / e.sum(axis=-1, keepdims=True)  # (bz, 1024, 21)

    roi_f = roi.astype(np.float32).reshape(bz, TOK, 1)
    n_roi = roi_f.sum()
    loss = (roi_f * np.abs(prob - aff)).sum()
    if n_roi > 0:
        loss = loss / n_roi
    return np.asarray(loss, dtype=np.float32)
